# revision 1
# baseline (speedup 1.0000x reference)
# kernel.py — MABSINK (Sinkhorn attention block) Trainium2 Bass kernel.
# Self-contained: hardcodes shapes B=8, n=1024, dQ=dV=512, H=8; shards batch
# across 8 NeuronCores (1 batch element per core), runs SPMD, gathers output.
#
# Math (per core, per head h; Q_h = (Q @ Wq.T + bq)[:, h*64:(h+1)*64]):
#   S   = Q_h Q_h^T / sqrt(512)            (symmetric!)
#   E   = exp(S);  r_i = sum_j E_ij;  c_j = sum_i E_ij / r_i
#   A   = n*mu' * E_ij / (r_i c_j),  mu' = 1/n + 1e-8
#   O_h = Q_h + A @ Q_h
# then head-recombine -> LN0 -> x + relu(x@Wo.T+bo) -> LN1.
#
# Layout trick: store E chunks as [a-part, b-free]; symmetry lets the same
# buffer serve as E^T. Fusions: exp+rowsum via ACT accum_out; Wt=E*invr and
# colsum c via one tensor_tensor_reduce; invc folded into the A@Q stationary
# operand; invr folded into Wt so (A@Q)^T comes straight out of the matmul.

import math
import numpy as np

B, N, DQ, DV, H = 8, 1024, 512, 512, 8
D = DV // H          # 64 head dim
P = 128
NRC = N // P         # 8 row chunks
NCC = DV // P        # 4 feature chunks
LN_EPS = 1e-5
SCALE_S = 1.0 / math.sqrt(DV)
AFACT = N * (1.0 / N + 1e-8)   # n * mu'

_CACHE = {}


def _build(mm_bf16=True, reps=1):
    import concourse.mybir as mybir
    from concourse import bacc
    import concourse.tile as tile
    from concourse.masks import make_identity
    from contextlib import ExitStack

    f32 = mybir.dt.float32
    bf = mybir.dt.bfloat16 if mm_bf16 else mybir.dt.float32
    AF = mybir.ActivationFunctionType
    OP = mybir.AluOpType
    AX = mybir.AxisListType

    nc = bacc.Bacc()
    dQ = nc.dram_tensor("Q", [N, DQ], f32, kind="ExternalInput")
    dWq = nc.dram_tensor("Wq", [DV, DQ], f32, kind="ExternalInput")
    dbq = nc.dram_tensor("bq", [DQ], f32, kind="ExternalInput")
    dWo = nc.dram_tensor("Wo", [DV, DV], f32, kind="ExternalInput")
    dbo = nc.dram_tensor("bo", [DV], f32, kind="ExternalInput")
    dg0 = nc.dram_tensor("g0", [DV], f32, kind="ExternalInput")
    db0 = nc.dram_tensor("b0", [DV], f32, kind="ExternalInput")
    dg1 = nc.dram_tensor("g1", [DV], f32, kind="ExternalInput")
    db1 = nc.dram_tensor("b1", [DV], f32, kind="ExternalInput")
    dout = nc.dram_tensor("out", [N, DV], f32, kind="ExternalOutput")

    with tile.TileContext(nc) as tc, ExitStack() as ctx:
        pc = ctx.enter_context(tc.tile_pool(name="pc", bufs=1))
        pq = ctx.enter_context(tc.tile_pool(name="pq", bufs=2))
        pqt = ctx.enter_context(tc.tile_pool(name="pqt", bufs=4))
        pw = ctx.enter_context(tc.tile_pool(name="pw", bufs=4))
        pqp = ctx.enter_context(tc.tile_pool(name="pqp", bufs=8))
        pqpt4 = ctx.enter_context(tc.tile_pool(name="pqpt4", bufs=4))
        pqptb = ctx.enter_context(tc.tile_pool(name="pqptb", bufs=2))
        pE = ctx.enter_context(tc.tile_pool(name="pE", bufs=8))
        pWt = ctx.enter_context(tc.tile_pool(name="pWt", bufs=9))
        pot = ctx.enter_context(tc.tile_pool(name="pot", bufs=4))
        po2 = ctx.enter_context(tc.tile_pool(name="po2", bufs=4))
        psqh = ctx.enter_context(tc.tile_pool(name="psqh", bufs=3))
        pstat = ctx.enter_context(tc.tile_pool(name="pstat", bufs=1))
        po1 = ctx.enter_context(tc.tile_pool(name="po1", bufs=4))
        psm = ctx.enter_context(tc.tile_pool(name="psm", bufs=2))   # small per-head
        pout = ctx.enter_context(tc.tile_pool(name="pout", bufs=2))

        ps_a = ctx.enter_context(tc.tile_pool(name="ps_a", bufs=3, space="PSUM"))
        ps_b = ctx.enter_context(tc.tile_pool(name="ps_b", bufs=2, space="PSUM"))
        ps_c = ctx.enter_context(tc.tile_pool(name="ps_c", bufs=3, space="PSUM"))

        # ---- constants -------------------------------------------------
        ident = pc.tile([P, P], f32, tag="ident")
        make_identity(nc, ident)
        ones_f = pc.tile([P, P], f32, tag="ones_f")
        nc.vector.memset(ones_f, 1.0)
        ones_b = pc.tile([1, P], bf, tag="ones_b")
        nc.vector.memset(ones_b, 1.0)
        zero_col = pc.tile([P, 1], f32, tag="zero_col")
        nc.vector.memset(zero_col, 0.0)
        eps_col = pc.tile([P, 1], f32, tag="eps_col")
        nc.vector.memset(eps_col, LN_EPS)
        nc.const_aps.aps[(f32, 0.0)] = zero_col
        nc.const_aps.aps[(f32, LN_EPS)] = eps_col
        # SEL[p, c*128+m] = (p == c): replicates row c of an [8,128] rhs
        # across all 128 output partitions via matmul.
        sel = pc.tile([NRC, NRC * P], bf, tag="sel")
        nc.gpsimd.memset(sel, 0.0)
        nc.gpsimd.affine_select(
            out=sel.rearrange("p (c m) -> p c m", m=P),
            in_=sel.rearrange("p (c m) -> p c m", m=P),
            compare_op=mybir.AluOpType.not_equal,
            fill=1.0, base=0,
            # affine = x - c  (x = partition, c = outer free dim): 0 on "diag"
            pattern=[[-1, NRC], [0, P]],
            channel_multiplier=1,
        )

        # per-partition column layouts [128, 4] (col cc = feature chunk cc)
        def col_vec(dvec, tag):
            v4 = pc.tile([NCC, P], f32, tag=tag + "4")
            nc.sync.dma_start(v4, dvec.rearrange("(c p) -> c p", p=P))
            pst = ps_a.tile([P, DV], f32, tag="s_ps", name="pst")
            nc.tensor.transpose(pst[:, :NCC], v4, ident[:NCC, :NCC])
            col = pc.tile([P, NCC], f32, tag=tag + "c")
            nc.scalar.activation(col, pst[:, :NCC], AF.Copy)
            return col

        bq_col = col_vec(dbq, "bq")
        g0_col = col_vec(dg0, "g0")
        b0_col = col_vec(db0, "b0")
        bo_col = col_vec(dbo, "bo")

        # replicated row layouts [128, 512] (same row on every partition)
        def repl_vec(dvec, tag):
            row = pc.tile([1, DV], f32, tag=tag + "r")
            nc.sync.dma_start(row, dvec[None])
            ps = ps_c.tile([P, DV], f32, tag="big", name="repl_ps")
            nc.tensor.matmul(ps, ones_f[:1, :], row, start=True, stop=True)
            rep = pc.tile([P, DV], f32, tag=tag + "rep")
            nc.scalar.activation(rep, ps, AF.Copy)
            return rep

        bq_rep = repl_vec(dbq, "bqv")
        g1_rep = repl_vec(dg1, "g1v")
        b1_rep = repl_vec(db1, "b1v")

        for _rep in range(reps):
            # ---- load Q, Wq, Wo; transpose via PE --------------------------
            # QT [k-part(4 tiles), r-free 1024] bf ; WqT/WoT [k-part, c-free 512] bf
            QT = [pqt.tile([P, N], bf, tag="qt", name="qt") for _ in range(NCC)]
            WqT = [pw.tile([P, DV], bf, tag="wqt", name="wqt") for _ in range(NCC)]
            WoT = [pw.tile([P, DV], bf, tag="wot", name="wot") for _ in range(NCC)]

            for rc in range(NRC):
                qsb = pq.tile([P, DQ], f32, tag="qsb")
                nc.sync.dma_start(qsb, dQ[rc * P:(rc + 1) * P, :])
                for kc in range(NCC):
                    pst = ps_a.tile([P, DV], f32, tag="s_ps", name="pst")
                    nc.tensor.transpose(pst[:, :P], qsb[:, kc * P:(kc + 1) * P], ident)
                    nc.scalar.activation(QT[kc][:, rc * P:(rc + 1) * P], pst[:, :P], AF.Copy)

            for src, dst in ((dWq, WqT), (dWo, WoT)):
                for rc in range(NCC):
                    wsb = pq.tile([P, DQ], f32, tag="qsb")
                    nc.sync.dma_start(wsb, src[rc * P:(rc + 1) * P, :])
                    for kc in range(NCC):
                        pst = ps_a.tile([P, DV], f32, tag="s_ps", name="pst")
                        nc.tensor.transpose(pst[:, :P], wsb[:, kc * P:(kc + 1) * P], ident)
                        nc.scalar.activation(dst[kc][:, rc * P:(rc + 1) * P], pst[:, :P], AF.Copy)

            # ---- Qp (row-major, bf) and QpT (transposed, f32 + bf) ---------
            Qp = [pqp.tile([P, DV], bf, tag="qp", name="qp") for _ in range(NRC)]
            for rc in range(NRC):
                ps = ps_c.tile([P, DV], f32, tag="big", name="qp_ps")
                for kc in range(NCC):
                    nc.tensor.matmul(ps, QT[kc][:, rc * P:(rc + 1) * P], WqT[kc],
                                     start=(kc == 0), stop=(kc == NCC - 1))
                nc.vector.tensor_tensor(Qp[rc], ps, bq_rep, OP.add)

            QpT = [pqpt4.tile([P, N], f32, tag="qpt", name="qpt") for _ in range(NCC)]
            QpTb = [pqptb.tile([P, N], bf, tag="qptb", name="qptb") for _ in range(NCC)]
            for cc in range(NCC):
                for hf in range(2):
                    ps = ps_c.tile([P, DV], f32, tag="big", name="qpt_ps")
                    for kc in range(NCC):
                        nc.tensor.matmul(
                            ps, WqT[kc][:, cc * P:(cc + 1) * P],
                            QT[kc][:, hf * DV:(hf + 1) * DV],
                            start=(kc == 0), stop=(kc == NCC - 1))
                    nc.scalar.activation(QpT[cc][:, hf * DV:(hf + 1) * DV], ps,
                                         AF.Identity, bias=bq_col[:, cc:cc + 1])
                nc.vector.tensor_copy(QpTb[cc], QpT[cc])

            # ---- OT accumulator (transposed head outputs + residual) -------
            OT = [pot.tile([P, N], f32, tag="ot", name="ot") for _ in range(NCC)]

            # ---- per-head Sinkhorn attention -------------------------------
            for h in range(H):
                tb = h // 2
                po = (h % 2) * D
                # Q_h^T as [64, 1024] slice of QpTb
                qht = QpTb[tb][po:po + D, :]

                E = [pE.tile([P, N], bf, tag="E", name="E") for _ in range(NRC)]
                r2 = psm.tile([P, 2 * NRC], f32, tag="r2")
                for ci in range(NRC):
                    for hf in range(2):
                        ps = ps_a.tile([P, DV], f32, tag="s_ps")
                        nc.tensor.matmul(ps, qht[:, ci * P:(ci + 1) * P],
                                         qht[:, hf * DV:(hf + 1) * DV],
                                         start=True, stop=True)
                        nc.scalar.activation(E[ci][:, hf * DV:(hf + 1) * DV], ps,
                                             AF.Exp, scale=SCALE_S,
                                             accum_out=r2[:, 2 * ci + hf:2 * ci + hf + 1])

                # r (partition layout) -> invr -> free layout -> replicate
                r2v = r2.rearrange("p (c two) -> p c two", two=2)
                r_mat = psm.tile([P, NRC], f32, tag="r_mat")
                nc.vector.tensor_tensor(r_mat, r2v[:, :, 0], r2v[:, :, 1], OP.add)
                invr = psm.tile([P, NRC], f32, tag="invr")
                nc.vector.reciprocal(invr, r_mat)
                pst = ps_a.tile([P, DV], f32, tag="s_ps", name="pst")
                nc.tensor.transpose(pst[:NRC, :P], invr, ident)
                sbt = psm.tile([NRC, P], bf, tag="sbt")
                nc.scalar.activation(sbt, pst[:NRC, :P], AF.Copy)
                rep_ps = [ps_c.tile([P, DV], f32, tag="big", name="repl_ps") for _ in range(2)]
                for c in range(NRC):
                    nc.tensor.matmul(rep_ps[c // 4][:, (c % 4) * P:(c % 4 + 1) * P],
                                     sel[:, c * P:(c + 1) * P], sbt,
                                     start=True, stop=True)
                invr_rep = psm.tile([P, N], bf, tag="invr_rep")
                for hf in range(2):
                    nc.scalar.activation(invr_rep[:, hf * DV:(hf + 1) * DV],
                                         rep_ps[hf], AF.Copy)

                # Wt = E * invr (free-wise) ; c = rowsum(Wt)  — one DVE op/chunk
                Wt = [pWt.tile([P, N], bf, tag="wt", name="wt") for _ in range(NRC)]
                c_mat = psm.tile([P, NRC], f32, tag="c_mat")
                for ci in range(NRC):
                    nc.vector.scalar_tensor_tensor(
                        Wt[ci], E[ci], 1.0, invr_rep, OP.mult, OP.mult,
                        accum_out=c_mat[:, ci:ci + 1])
                invc = psm.tile([P, NRC], f32, tag="invc")
                nc.vector.reciprocal(invc, c_mat)

                # Qc = Qp_head * invc * AFACT  (per-partition scale)
                Qc = [psm.tile([P, D], bf, tag=f"qc{jc}", name=f"qc{jc}") for jc in range(NRC)]
                for jc in range(NRC):
                    nc.vector.tensor_scalar(Qc[jc], Qp[jc][:, h * D:(h + 1) * D],
                                            invc[:, jc:jc + 1], AFACT, OP.mult, OP.mult)

                # O_h^T = sum_j Qc[j,:]^T Wt[j,:]  -> [64, 1024]; + residual
                for hf in range(2):
                    ps = ps_b.tile([D, DV], f32, tag="aq_ps")
                    for jc in range(NRC):
                        nc.tensor.matmul(ps, Qc[jc], Wt[jc][:, hf * DV:(hf + 1) * DV],
                                         start=(jc == 0), stop=(jc == NRC - 1))
                    nc.vector.tensor_tensor(
                        OT[tb][po:po + D, hf * DV:(hf + 1) * DV], ps,
                        QpT[tb][po:po + D, hf * DV:(hf + 1) * DV], OP.add)

            # ---- LN0 (transposed: stats over partitions via ones-matmul) ---
            m_rep = pstat.tile([P, N], f32, tag="m_rep")
            var_rep = pstat.tile([P, N], f32, tag="var_rep")
            rstd_rep = pstat.tile([P, N], f32, tag="rstd_rep")
            for hf in range(2):
                sl = slice(hf * DV, (hf + 1) * DV)
                sps = ps_c.tile([P, DV], f32, tag="big", name="ln0s")
                qps = ps_c.tile([P, DV], f32, tag="big", name="ln0q")
                for cc in range(NCC):
                    nc.tensor.matmul(sps, ones_f, OT[cc][:, hf * DV:(hf + 1) * DV],
                                     start=(cc == 0), stop=(cc == NCC - 1))
                for cc in range(NCC):
                    sqh = psqh.tile([P, DV], f32, tag="sqh", name="sqh")
                    nc.scalar.activation(sqh, OT[cc][:, hf * DV:(hf + 1) * DV], AF.Square)
                    nc.tensor.matmul(qps, ones_f, sqh,
                                     start=(cc == 0), stop=(cc == NCC - 1))
                nc.vector.tensor_scalar_mul(m_rep[:, sl], sps, 1.0 / DV)
                nc.vector.tensor_scalar_mul(var_rep[:, sl], qps, 1.0 / DV)
            t_rep = pstat.tile([P, N], f32, tag="t_rep")
            nc.vector.tensor_tensor(t_rep, m_rep, m_rep, OP.mult)
            nc.vector.tensor_tensor(var_rep, var_rep, t_rep, OP.subtract)
            nc.scalar.activation(t_rep, var_rep, AF.Sqrt, bias=LN_EPS)
            nc.vector.reciprocal(rstd_rep, t_rep)

            O1T = [po1.tile([P, N], f32, tag="o1t", name="o1t") for _ in range(NCC)]
            O1Tb = [pWt.tile([P, N], bf, tag="wt", name="wt") for _ in range(NCC)]
            for cc in range(NCC):
                nc.vector.tensor_tensor(O1T[cc], OT[cc], m_rep, OP.subtract)
                nc.vector.tensor_tensor(O1T[cc], O1T[cc], rstd_rep, OP.mult)
                nc.vector.tensor_scalar(O1T[cc], O1T[cc], g0_col[:, cc:cc + 1],
                                        b0_col[:, cc:cc + 1], OP.mult, OP.add)
                nc.scalar.activation(O1Tb[cc], O1T[cc], AF.Copy)

            # ---- FFN: O2T = O1T + relu(Wo @ O1T + bo) ----------------------
            O2T = [po2.tile([P, N], f32, tag="o2t", name="o2t") for _ in range(NCC)]
            for c2 in range(NCC):
                for hf in range(2):
                    ps = ps_c.tile([P, DV], f32, tag="big", name="ffn_ps")
                    for cc in range(NCC):
                        nc.tensor.matmul(ps, WoT[cc][:, c2 * P:(c2 + 1) * P],
                                         O1Tb[cc][:, hf * DV:(hf + 1) * DV],
                                         start=(cc == 0), stop=(cc == NCC - 1))
                    nc.scalar.activation(ps, ps, AF.Relu, bias=bo_col[:, c2:c2 + 1])
                    nc.vector.tensor_tensor(O2T[c2][:, hf * DV:(hf + 1) * DV], ps,
                                            O1T[c2][:, hf * DV:(hf + 1) * DV], OP.add)

            # ---- LN1 (row-major after PE transpose) + store ----------------
            for rc in range(NRC):
                psf = ps_c.tile([P, DV], f32, tag="big", name="ln1_ps")
                for cc in range(NCC):
                    nc.tensor.transpose(psf[:, cc * P:(cc + 1) * P],
                                        O2T[cc][:, rc * P:(rc + 1) * P], ident)
                mean = psm.tile([P, 1], f32, tag="ln1_mean")
                nc.vector.tensor_reduce(mean, psf, AX.X, OP.add)
                nc.vector.tensor_scalar_mul(mean, mean, 1.0 / DV)
                xc = pout.tile([P, DV], f32, tag="ln1_xc")
                nc.vector.tensor_scalar_sub(xc, psf, mean)
                sqj = pout.tile([P, DV], bf, tag="ln1_sqj")
                ss = psm.tile([P, 1], f32, tag="ln1_ss")
                nc.scalar.activation(sqj, xc, AF.Square, accum_out=ss)
                nc.vector.tensor_scalar_mul(ss, ss, 1.0 / DV)
                sd = psm.tile([P, 1], f32, tag="ln1_sd")
                nc.scalar.activation(sd, ss, AF.Sqrt, bias=LN_EPS)
                rstd = psm.tile([P, 1], f32, tag="ln1_rstd")
                nc.vector.reciprocal(rstd, sd)
                ob = pout.tile([P, DV], f32, tag="ln1_out")
                nc.vector.scalar_tensor_tensor(ob, xc, rstd, g1_rep, OP.mult, OP.mult)
                nc.vector.tensor_tensor(ob, ob, b1_rep, OP.add)
                nc.sync.dma_start(dout[rc * P:(rc + 1) * P, :], ob)

    nc.finalize()
    return nc


def kernel(**inputs):
    from concourse.bass_utils import run_bass_kernel_spmd

    if "nc" not in _CACHE:
        _CACHE["nc"] = _build()
    nc = _CACHE["nc"]

    Q = np.ascontiguousarray(np.asarray(inputs["Q"], dtype=np.float32))
    shared = {k: np.ascontiguousarray(np.asarray(inputs[k], dtype=np.float32))
              for k in ("Wq", "bq", "Wo", "bo", "g0", "b0", "g1", "b1")}
    in_maps = [dict(Q=np.ascontiguousarray(Q[b]), **shared) for b in range(B)]

    res = run_bass_kernel_spmd(nc, in_maps, core_ids=list(range(B)),
                               **_CACHE.get("run_kwargs", {}))
    _CACHE["last_result"] = res
    return np.stack([r["out"] for r in res.results], axis=0)

